# revision 1
# baseline (speedup 1.0000x reference)
"""Relative-position attention (TransformerXL-style) on 8 TRN2 NeuronCores.

Sharding: data-parallel over batch (b=8 -> 1 batch element per core); weights
replicated. No collectives needed.

Per-core pipeline (n=1024, dim=512, heads=8, d_head=64):
  qT = Wq^T x^T, kT = Wk^T x^T   [inner, n]   (bf16 matmuls, fp32 psum)
  v  = x Wv                      [n, inner]
  per (head h, 128-row query tile m):
    S_psum[128, 1024]  = qTh_m^T kTh           (2 matmuls)
    T_psum[128, 1151]  = qTh_m^T relT[:, off_m:off_m+1151]  (3 matmuls)
    T_sb (bf16) <- T_psum ; pos_sb[p, j] = T_sb[p, j + 127 - p]  (diagonal
        SBUF->SBUF DMA: the Toeplitz/rel-shift gather)
    S_psum += pos_sb (DVE, in-place in PSUM)
    P_sb (bf16), Z = exp(0.125 * S_psum) with fused row-sum   (ACT)
    diag = identity * (1/Z)  (per-partition scalar mul -> diag(r))
    PT_jb = P[:, jb]^T @ diag(r)   (PE transpose; folds softmax normalization)
    OT_psum[64, 128] += v_jb_h^T @ PT_jb       (8 matmuls)
  out_m[128, 512] = OT_m^T Wo + ones^T bo      (5 matmuls, K=1 bias trick)

The rel-pos table is host-preprocessed into relT[d, c] = rel_emb[1024 -
clip(c - 511, 0, 1024), d] so that pos_attn[i, j] = (q_i . relT[:, j - i +
1023]) and clipping is baked into the padded table.
"""
import sys

sys.path.insert(0, "/opt/trn_rl_repo")

import numpy as np

import concourse.bass as bass
import concourse.bacc as bacc
import concourse.mybir as mybir
import concourse.tile as tile
from concourse.ap import AP
from concourse.bass_utils import run_bass_kernel_spmd

F32 = mybir.dt.float32
BF16 = mybir.dt.bfloat16

B, N, DIM = 8, 1024, 512
HEADS, DH = 8, 64
INNER = HEADS * DH
MAX_POS = 512
RELW = 2 * MAX_POS + 1        # 1025 rel-emb rows
RELTW = 2047                  # extended/clip-padded table width
TW = 1151                     # per-query-tile T width (1024 + 127)
TWPAD = 1152
KC = DIM // 128               # 4 contraction chunks
MT = N // 128                 # 8 query row tiles
SCALE = DH ** -0.5

_CACHE = {}

import os
CFG = {
    "asb": int(os.environ.get("K_ASB", "5")),
    "s": int(os.environ.get("K_S", "2")),
    "t": int(os.environ.get("K_T", "2")),
    "pt": int(os.environ.get("K_PT", "2")),
    "ot": int(os.environ.get("K_OT", "1")),
    "ptsb": int(os.environ.get("K_PTSB", "4")),
}


def _build_nc():
    nc = bacc.Bacc()
    xT_in = nc.declare_dram_parameter("xT", [DIM, N], BF16, isOutput=False)
    wq_in = nc.declare_dram_parameter("wq", [DIM, INNER], BF16, isOutput=False)
    wk_in = nc.declare_dram_parameter("wk", [DIM, INNER], BF16, isOutput=False)
    wv_in = nc.declare_dram_parameter("wv", [DIM, INNER], BF16, isOutput=False)
    wo_in = nc.declare_dram_parameter("wo", [INNER, DIM], BF16, isOutput=False)
    rel_in = nc.declare_dram_parameter("relT", [128, RELTW], BF16, isOutput=False)
    bo_in = nc.declare_dram_parameter("bo", [1, DIM], BF16, isOutput=False)
    ident_in = nc.declare_dram_parameter("ident", [128, 128], BF16, isOutput=False)
    out_ext = nc.declare_dram_parameter("out", [N, DIM], F32, isOutput=True)

    with tile.TileContext(nc) as tc:
        with tc.tile_pool(name="persist", bufs=1) as pp:
            # ---- load persistent operands ----
            xT_sb = [pp.tile([128, N], BF16, name=f"xT{k}") for k in range(KC)]
            wq_sb = [pp.tile([128, INNER], BF16, name=f"wq{k}") for k in range(KC)]
            wk_sb = [pp.tile([128, INNER], BF16, name=f"wk{k}") for k in range(KC)]
            wv_sb = [pp.tile([128, INNER], BF16, name=f"wv{k}") for k in range(KC)]
            wo_sb = [pp.tile([128, DIM], BF16, name=f"wo{k}") for k in range(KC)]
            rel_sb = pp.tile([128, RELTW], BF16)
            bo_sb = pp.tile([1, DIM], BF16)
            ident_sb = pp.tile([128, 128], BF16)
            ones_sb = pp.tile([1, 128], BF16)
            onesw_sb = pp.tile([128, 512], BF16)
            for k in range(KC):
                nc.sync.dma_start(out=xT_sb[k][:], in_=xT_in[128 * k:128 * (k + 1), :])
                nc.sync.dma_start(out=wq_sb[k][:], in_=wq_in[128 * k:128 * (k + 1), :])
                nc.sync.dma_start(out=wk_sb[k][:], in_=wk_in[128 * k:128 * (k + 1), :])
            for k in range(KC):
                nc.sync.dma_start(out=wv_sb[k][:], in_=wv_in[128 * k:128 * (k + 1), :])
            nc.sync.dma_start(out=rel_sb[:], in_=rel_in[:])
            nc.sync.dma_start(out=ident_sb[:], in_=ident_in[:])
            for k in range(KC):
                nc.sync.dma_start(out=wo_sb[k][:], in_=wo_in[128 * k:128 * (k + 1), :])
            nc.sync.dma_start(out=bo_sb[:], in_=bo_in[:])
            nc.gpsimd.memset(ones_sb[:], 1.0)
            nc.gpsimd.memset(onesw_sb[:], 1.0)

            # ---- projections ----
            qT_sb = [pp.tile([128, N], BF16, name=f"qT{t}") for t in range(KC)]
            kT_sb = [pp.tile([128, N], BF16, name=f"kT{t}") for t in range(KC)]
            v_sb = [pp.tile([128, INNER], BF16, name=f"v{t}") for t in range(MT)]
            oT_sb = [pp.tile([128, N], BF16, name=f"oT{t}") for t in range(KC)]

            with tc.tile_pool(name="proj_ps", bufs=4, space="PSUM") as proj_ps:
                for t in range(KC):          # qT / kT tiles: inner rows 128t..
                    for jc in range(2):      # n column chunks of 512
                        for which, w_sb, dst in (("q", wq_sb, qT_sb), ("k", wk_sb, kT_sb)):
                            ps = proj_ps.tile([128, 512], F32, tag="pps",
                                              name=f"ps{which}{t}{jc}")
                            for k in range(KC):
                                nc.tensor.matmul(
                                    ps[:],
                                    w_sb[k][:, 128 * t:128 * (t + 1)],
                                    xT_sb[k][:, 512 * jc:512 * (jc + 1)],
                                    start=(k == 0), stop=(k == KC - 1))
                            nc.vector.tensor_copy(dst[t][:, 512 * jc:512 * (jc + 1)], ps[:])
                for t in range(MT):          # v tiles: n rows 128t..
                    ps = proj_ps.tile([128, 512], F32, tag="pps", name=f"psv{t}")
                    for k in range(KC):
                        nc.tensor.matmul(
                            ps[:],
                            xT_sb[k][:, 128 * t:128 * (t + 1)],
                            wv_sb[k][:],
                            start=(k == 0), stop=(k == KC - 1))
                    if t % 2 == 0:
                        nc.scalar.copy(v_sb[t][:], ps[:])
                    else:
                        nc.vector.tensor_copy(v_sb[t][:], ps[:])

            # ---- attention ----
            with tc.tile_pool(name="attn_sb", bufs=CFG["asb"]) as asb, \
                 tc.tile_pool(name="attn_ps", bufs=CFG["s"], space="PSUM") as aps, \
                 tc.tile_pool(name="tp_ps", bufs=CFG["t"], space="PSUM") as tps, \
                 tc.tile_pool(name="pt_ps", bufs=CFG["pt"], space="PSUM") as ptps, \
                 tc.tile_pool(name="ot_ps", bufs=CFG["ot"], space="PSUM") as otps, \
                 tc.tile_pool(name="fin_sb", bufs=2) as osb, \
                 tc.tile_pool(name="fin_ps", bufs=1, space="PSUM") as ops:
                for m in range(MT):
                    for h in range(HEADS):
                        th, ph = h // 2, (h % 2) * 64
                        qh = qT_sb[th][ph:ph + 64, 128 * m:128 * (m + 1)]
                        s_ps = [aps.tile([128, 512], F32, tag="s_ps",
                                         name=f"s_ps{jc}") for jc in range(2)]
                        for jc in range(2):
                            nc.tensor.matmul(
                                s_ps[jc][:],
                                qh,
                                kT_sb[th][ph:ph + 64, 512 * jc:512 * (jc + 1)],
                                start=True, stop=False)
                        # rel-pos T tile: only the unclipped band of relT;
                        # the clipped tails are constant per row (edge cols).
                        off = 896 - 128 * m
                        lo = max(0, 128 * m - 385)
                        hi = min(1150, 128 * m + 639)
                        w = hi - lo + 1
                        t_sb = asb.tile([128, TWPAD], BF16, name="t_sb")
                        t_chunks = []
                        for ci, (c0, cw) in enumerate(((lo, 512), (lo + 512, w - 512))):
                            t_ps = tps.tile([128, 512], F32, tag="t_ps",
                                            name=f"t_ps{ci}")
                            t_chunks.append(t_ps)
                            nc.tensor.matmul(
                                t_ps[:, 0:cw],
                                qh,
                                rel_sb[ph:ph + 64, off + c0:off + c0 + cw],
                                start=True, stop=True)
                            nc.vector.tensor_copy(t_sb[:, c0:c0 + cw], t_ps[:, 0:cw])
                        if lo > 0:    # low clip tail: rows of rel_emb[1024]
                            nc.vector.tensor_scalar_mul(
                                t_sb[:, 0:lo], onesw_sb[:, 0:lo],
                                t_chunks[0][:, 0:1])
                        if hi < 1150:  # high clip tail: rows of rel_emb[0]
                            nc.vector.tensor_scalar_mul(
                                t_sb[:, hi + 1:1151], onesw_sb[:, 0:1150 - hi],
                                t_chunks[1][:, w - 513:w - 512])
                        # Toeplitz skew: pos[p, j] = t_sb[p, j + 127 - p]
                        pos_sb = asb.tile([128, N], BF16, name="pos_sb")
                        t_ap = t_sb
                        skew = AP(t_ap.tensor, t_ap.offset + 127,
                                  [[TWPAD - 1, 128], [1, N]])
                        nc.sync.dma_start(out=pos_sb[:], in_=skew)
                        # S += pos, accumulated on the PE via identity matmul
                        for jc in range(2):
                            nc.tensor.matmul(
                                s_ps[jc][:],
                                ident_sb[:],
                                pos_sb[:, 512 * jc:512 * (jc + 1)],
                                start=False, stop=True)
                        # softmax (no max-subtraction: logits are O(5))
                        p_sb = asb.tile([128, N], BF16, name="p_sb")
                        z_sb = [asb.tile([128, 1], F32, tag="z_sb",
                                         name=f"z_sb{jc}") for jc in range(2)]
                        for jc in range(2):
                            nc.scalar.activation(
                                p_sb[:, 512 * jc:512 * (jc + 1)], s_ps[jc][:],
                                mybir.ActivationFunctionType.Exp,
                                scale=SCALE, accum_out=z_sb[jc][:])
                        zt_sb = asb.tile([128, 1], F32, name="zt_sb")
                        nc.vector.tensor_add(zt_sb[:], z_sb[0][:], z_sb[1][:])
                        r_sb = asb.tile([128, 1], F32, name="r_sb")
                        nc.vector.reciprocal(r_sb[:], zt_sb[:])
                        diag_sb = asb.tile([128, 128], BF16, name="diag_sb")
                        nc.vector.tensor_scalar_mul(diag_sb[:], ident_sb[:], r_sb[:])
                        # P^T @ diag(r): 4 transposed blocks per PSUM bank tile
                        ot_ps = otps.tile([64, 128], F32, name="ot_ps")
                        for half in range(2):
                            pt_ps = ptps.tile([128, 512], F32, name="pt_ps")
                            for q in range(4):
                                jb = 4 * half + q
                                nc.tensor.matmul(
                                    pt_ps[:, 128 * q:128 * (q + 1)],
                                    p_sb[:, 128 * jb:128 * (jb + 1)],
                                    diag_sb[:], start=True, stop=True)
                            pt_sb = asb.tile([128, 512], BF16, name="pt_sb",
                                             bufs=CFG["ptsb"])
                            if half == 0:
                                nc.scalar.copy(pt_sb[:], pt_ps[:])
                            else:
                                nc.vector.tensor_copy(pt_sb[:], pt_ps[:])
                            for q in range(4):
                                jb = 4 * half + q
                                nc.tensor.matmul(
                                    ot_ps[:],
                                    v_sb[jb][:, DH * h:DH * (h + 1)],
                                    pt_sb[:, 128 * q:128 * (q + 1)],
                                    start=(jb == 0), stop=(jb == MT - 1))
                        nc.scalar.copy(
                            oT_sb[th][ph:ph + 64, 128 * m:128 * (m + 1)], ot_ps[:])

                    # ---- output projection for this query tile ----
                    o_ps = ops.tile([128, DIM], F32, name="o_ps")
                    for g in range(KC):
                        nc.tensor.matmul(
                            o_ps[:],
                            oT_sb[g][:, 128 * m:128 * (m + 1)],
                            wo_sb[g][:],
                            start=(g == 0), stop=False)
                    nc.tensor.matmul(o_ps[:], ones_sb[:], bo_sb[:],
                                     start=False, stop=True)
                    o_sb = osb.tile([128, DIM], F32, name="o_sb")
                    if m % 2 == 0:
                        nc.scalar.copy(o_sb[:], o_ps[:])
                    else:
                        nc.vector.tensor_copy(o_sb[:], o_ps[:])
                    nc.sync.dma_start(
                        out=out_ext[128 * m:128 * (m + 1), :], in_=o_sb[:])
    nc.compile()
    return nc


def _prep_inputs(x, Wq, Wkv, rel_emb, Wo, bo):
    bf = np.float32  # staging dtype before bf16 cast
    import ml_dtypes
    tobf = lambda a: np.asarray(a, dtype=np.float32).astype(ml_dtypes.bfloat16)
    Wk = Wkv[:, :INNER]
    Wv = Wkv[:, INNER:]
    # relT[d, c] = rel_emb[1024 - clip(c - 511, 0, 1024), d], duplicated onto
    # partitions 64..127 so both head-parity quadrants can read it.
    c = np.arange(RELTW)
    rows = RELW - 1 - np.clip(c - (MAX_POS - 1), 0, RELW - 1)
    relT64 = np.ascontiguousarray(rel_emb[rows].T)          # [64, 2047]
    relT = np.concatenate([relT64, relT64], axis=0)         # [128, 2047]
    ident = np.eye(128, dtype=np.float32)
    base = {
        "wq": tobf(Wq), "wk": tobf(Wk), "wv": tobf(Wv), "wo": tobf(Wo),
        "relT": tobf(relT), "bo": tobf(bo.reshape(1, DIM)),
        "ident": tobf(ident),
    }
    in_maps = []
    for c_ in range(B):
        m = dict(base)
        m["xT"] = tobf(np.ascontiguousarray(x[c_].T))
        in_maps.append(m)
    return in_maps


def kernel(x, Wq, Wkv, rel_emb, Wo, bo):
    if "nc" not in _CACHE:
        _CACHE["nc"] = _build_nc()
    nc = _CACHE["nc"]
    in_maps = _prep_inputs(x, Wq, Wkv, rel_emb, Wo, bo)
    res = run_bass_kernel_spmd(nc, in_maps, list(range(B))).results
    out = np.stack([res[c]["out"] for c in range(B)]).astype(np.float32)
    return out

